# revision 9
# baseline (speedup 1.0000x reference)
"""Plackett-Luce NLL loss kernel for Trainium2 (Bass/Tile), 8-core data parallel.

Math (per race, N=18 horses):
  m      = mask (0/1); k = sum(m)
  key_h  = rank_h * 16 + (h & 15)          (fp32-exact composite sort key)
  M      = max_h(score_h * m_h)            (safe shift, >= max valid score or 0)
  E_h    = exp(score_h - M) * m_h
  S_c    = E_c + sum_h [key_h > key_c] * E_h   (suffix sum in finish order)
  loss   = sum_{c: m_c=1} (M + log(S_c) - score_c)
The last-place valid horse's term is log(exp(s-M)) + M - s ~= 0, matching the
reference which excludes it; k<=1 races contribute ~0. Races are sharded
across 8 cores; each core emits per-lane partial loss sums and valid-race
counts; the host reduces and divides.
"""

import os

import numpy as np

B_TOTAL = 1048576
N = 18
N_CORES = 8
R_CORE = B_TOTAL // N_CORES  # 131072 races per core
P = 128                      # SBUF partitions
F = 32                       # races per partition per tile
RACES_PER_TILE = P * F       # 4096
N_TILES = R_CORE // RACES_PER_TILE


def build_kernel(r_core=R_CORE):
    import concourse.bacc as bacc
    import concourse.mybir as mybir
    from concourse.tile import TileContext

    f32 = mybir.dt.float32
    i32 = mybir.dt.int32
    ALU = mybir.AluOpType
    AF = mybir.ActivationFunctionType

    n_tiles = r_core // RACES_PER_TILE
    assert n_tiles * RACES_PER_TILE == r_core

    nc = bacc.Bacc()
    sc = nc.declare_dram_parameter("scores", [r_core, N], f32, isOutput=False)
    rk = nc.declare_dram_parameter("rankings", [r_core, N], i32, isOutput=False)
    mk = nc.declare_dram_parameter("mask", [r_core, N], i32, isOutput=False)
    out = nc.declare_dram_parameter("out", [P, 2], f32, isOutput=True)

    with TileContext(nc) as tc:
        with (
            tc.tile_pool(name="consts", bufs=1) as cpool,
            tc.tile_pool(name="io", bufs=3) as iop,
            tc.tile_pool(name="work", bufs=2) as wp,
            tc.tile_pool(name="big", bufs=2) as bigp,
            tc.tile_pool(name="accs", bufs=1) as accp,
        ):
            # constant column-index (h & 15) pattern, one value per horse slot
            idx16 = cpool.tile([P, N], f32)
            for h in range(N):
                nc.vector.memset(idx16[:, h : h + 1], float(h & 15))
            idx16_b = idx16[:, :].unsqueeze(1).broadcast_to([P, F, N])

            loss_acc = accp.tile([P, F], f32)
            valid_acc = accp.tile([P, F], f32)
            nc.vector.memset(loss_acc, 0.0)
            nc.vector.memset(valid_acc, 0.0)

            for t in range(n_tiles):
                lo = t * RACES_PER_TILE
                hi = lo + RACES_PER_TILE
                s_raw = iop.tile([P, F * N], f32, tag="s")
                r_raw = iop.tile([P, F * N], i32, tag="r")
                m_raw = iop.tile([P, F * N], i32, tag="m")
                nc.sync.dma_start(
                    out=s_raw, in_=sc[lo:hi, :].rearrange("(p f) h -> p (f h)", p=P)
                )
                nc.sync.dma_start(
                    out=r_raw, in_=rk[lo:hi, :].rearrange("(p f) h -> p (f h)", p=P)
                )
                nc.sync.dma_start(
                    out=m_raw, in_=mk[lo:hi, :].rearrange("(p f) h -> p (f h)", p=P)
                )

                m_f = wp.tile([P, F * N], f32, tag="mf")
                key = wp.tile([P, F * N], f32, tag="key")
                sm = wp.tile([P, F * N], f32, tag="sm")
                E = wp.tile([P, F * N], f32, tag="E")
                Sb = wp.tile([P, F * N], f32, tag="Sb")
                M2 = wp.tile([P, F], f32, tag="M2")
                kcnt = wp.tile([P, F], f32, tag="kcnt")
                p1 = wp.tile([P, F], f32, tag="p1")
                tmpF = wp.tile([P, F], f32, tag="tmpF")
                cmpE = bigp.tile([P, F * N * N], f32, tag="cmpE")

                m_f3 = m_f[:, :].rearrange("p (f h) -> p f h", h=N)
                key3 = key[:, :].rearrange("p (f h) -> p f h", h=N)
                E3 = E[:, :].rearrange("p (f h) -> p f h", h=N)
                sm3 = sm[:, :].rearrange("p (f h) -> p f h", h=N)
                Sb3 = Sb[:, :].rearrange("p (f h) -> p f h", h=N)
                cmp4 = cmpE[:, :].rearrange("p (f c h) -> p f c h", c=N, h=N)

                # int -> float converts (sole readers of the DMA'd tiles)
                nc.vector.tensor_copy(out=m_f[:, :], in_=m_raw[:, :])
                nc.vector.tensor_copy(out=key[:, :], in_=r_raw[:, :])
                # key = key*16 + idx16
                nc.vector.scalar_tensor_tensor(
                    out=key3,
                    in0=key3,
                    scalar=16.0,
                    in1=idx16_b,
                    op0=ALU.mult,
                    op1=ALU.add,
                )
                # sm = s*m (sole reader of s_raw; equals s on valid slots)
                nc.vector.tensor_tensor(
                    out=sm[:, :], in0=s_raw[:, :], in1=m_f[:, :], op=ALU.mult
                )
                # M2 = max over race of sm
                nc.vector.tensor_reduce(
                    out=M2[:, :], in_=sm3, axis=mybir.AxisListType.X, op=ALU.max
                )
                # x = sm - M2  (into E buffer)
                nc.vector.tensor_tensor(
                    out=E3,
                    in0=sm3,
                    in1=M2[:, :].unsqueeze(2).broadcast_to([P, F, N]),
                    op=ALU.subtract,
                )
                # E = exp(x) on scalar engine
                nc.scalar.activation(out=E[:, :], in_=E[:, :], func=AF.Exp)
                # E *= m
                nc.vector.tensor_tensor(
                    out=E[:, :], in0=E[:, :], in1=m_f[:, :], op=ALU.mult
                )

                # --- the O(N^2) comparison block (single big ops) ---
                keyH = key3.unsqueeze(2).broadcast_to([P, F, N, N])  # varies over h
                keyC = key3.unsqueeze(3).broadcast_to([P, F, N, N])  # varies over c
                EH = E3.unsqueeze(2).broadcast_to([P, F, N, N])
                nc.vector.tensor_tensor(out=cmp4, in0=keyH, in1=keyC, op=ALU.is_gt)
                nc.vector.tensor_tensor(out=cmp4, in0=cmp4, in1=EH, op=ALU.mult)
                nc.vector.tensor_reduce(
                    out=Sb3, in_=cmp4, axis=mybir.AxisListType.X, op=ALU.add
                )
                # S += E  (diagonal term)
                nc.vector.tensor_tensor(
                    out=Sb[:, :], in0=Sb[:, :], in1=E[:, :], op=ALU.add
                )
                # S = max(S, tiny) to keep log finite on masked slots
                nc.vector.tensor_scalar_max(Sb[:, :], Sb[:, :], 1e-30)
                # logS
                nc.scalar.activation(out=Sb[:, :], in_=Sb[:, :], func=AF.Ln)
                # d = logS - s ; dm = d*m  (sm == s on valid slots)
                nc.vector.tensor_tensor(
                    out=Sb[:, :], in0=Sb[:, :], in1=sm[:, :], op=ALU.subtract
                )
                nc.vector.tensor_tensor(
                    out=Sb[:, :], in0=Sb[:, :], in1=m_f[:, :], op=ALU.mult
                )
                # p1 = sum over race of dm ; k = sum over race of m
                nc.vector.tensor_reduce(
                    out=p1[:, :], in_=Sb3, axis=mybir.AxisListType.X, op=ALU.add
                )
                nc.vector.tensor_reduce(
                    out=kcnt[:, :], in_=m_f3, axis=mybir.AxisListType.X, op=ALU.add
                )
                # loss_acc += p1 + k*M2
                nc.vector.tensor_tensor(
                    out=tmpF[:, :], in0=kcnt[:, :], in1=M2[:, :], op=ALU.mult
                )
                nc.vector.tensor_tensor(
                    out=tmpF[:, :], in0=tmpF[:, :], in1=p1[:, :], op=ALU.add
                )
                nc.vector.tensor_tensor(
                    out=loss_acc[:, :], in0=loss_acc[:, :], in1=tmpF[:, :], op=ALU.add
                )
                # valid_acc += (k > 1)
                nc.vector.tensor_scalar(
                    out=tmpF[:, :],
                    in0=kcnt[:, :],
                    scalar1=1.0,
                    scalar2=None,
                    op0=ALU.is_gt,
                )
                nc.vector.tensor_tensor(
                    out=valid_acc[:, :],
                    in0=valid_acc[:, :],
                    in1=tmpF[:, :],
                    op=ALU.add,
                )

            # final per-lane reduction and store
            res = accp.tile([P, 2], f32)
            nc.vector.tensor_reduce(
                out=res[:, 0:1],
                in_=loss_acc[:, :],
                axis=mybir.AxisListType.X,
                op=ALU.add,
            )
            nc.vector.tensor_reduce(
                out=res[:, 1:2],
                in_=valid_acc[:, :],
                axis=mybir.AxisListType.X,
                op=ALU.add,
            )
            nc.sync.dma_start(out=out[:, :], in_=res[:, :])

    nc.compile()
    return nc


_NC_CACHE = None


def _get_nc():
    global _NC_CACHE
    if _NC_CACHE is None:
        _NC_CACHE = build_kernel()
    return _NC_CACHE


def kernel(scores, rankings, mask):
    from concourse.bass_utils import run_bass_kernel_spmd

    scores = np.ascontiguousarray(np.asarray(scores, dtype=np.float32))
    rankings = np.ascontiguousarray(np.asarray(rankings, dtype=np.int32))
    mask = np.ascontiguousarray(np.asarray(mask, dtype=np.int32))
    assert scores.shape == (B_TOTAL, N)

    nc = _get_nc()
    in_maps = []
    for c in range(N_CORES):
        lo, hi = c * R_CORE, (c + 1) * R_CORE
        in_maps.append(
            {
                "scores": scores[lo:hi],
                "rankings": rankings[lo:hi],
                "mask": mask[lo:hi],
            }
        )
    trace = bool(int(os.environ.get("PL_TRACE", "0")))
    res = run_bass_kernel_spmd(
        nc, in_maps, core_ids=list(range(N_CORES)), trace=trace
    )
    if trace and res.exec_time_ns is not None:
        print(f"HW exec time: {res.exec_time_ns} ns")
        if res.instructions_and_trace is not None:
            print(f"trace: {res.instructions_and_trace[1]}")

    total = np.float64(0.0)
    count = np.float64(0.0)
    for r in res.results:
        o = r["out"].astype(np.float64)
        total += o[:, 0].sum()
        count += o[:, 1].sum()
    denom = max(count, 1.0)
    return np.float32(total / denom)
